# revision 39
# baseline (speedup 1.0000x reference)
"""Trainium2 Bass kernel for nn_AlltagCopyCtxGenerator (moe_routing).

Expert-parallel sharding with host-side token shuffle (no collectives):
  - Tokens are assigned to cores BY POS GROUP (MoE expert dispatch done on the
    host): each core owns the tokens of the <=2 POS decoders it hosts, plus a
    share of the non-decoded (pos>=12) tokens. Each core therefore reads only
    its own ~2 decoder weight matrices (4MB instead of 24MB).
  - Host: bit-exact jax-CPU reproduction of the gumbel noise streams and the
    tiny copy-gate head (whose `==1.0` / int-cast semantics depend on exact
    float bits), routing tables, packed per-POS gather tables
    [psr_row | atk_row | word], weight/ctx layout transforms.
  - Device (per core, SPMD, raw bass): per decode unit (128 routed tokens),
    f32 matmul against the unit's dec_W (+rank-1 bias), fused exp/accum for
    entropy, gumbel argmax via max/max_index, indirect-DMA gather of packed
    embedding rows, then the copy-gate blend psr/atk = ori*c0 + obf*c1.
  - Host epilogue: inverse shuffle, luck-exact out_word/obf_char/masks/losses.
"""
import os
import sys
import types
import numpy as np

# ---------------- problem constants (hardcoded per spec) ----------------
B, L, HS = 32, 128, 512
P, M = 12, 1024
V, D = 50000, 128
CHAR_LEN = 16
N = B * L
NCORES = 8
SLAB = 260  # psr[128] | atk[128] | word | pad
KC = HS // 128  # contraction chunks
NSLOT = 4       # weight slots per core

LAST_EXEC_NS = None  # filled by kernel() when KERNEL_TRACE is set


def _install_axon_profile_hook():
    """Provide antenv.axon_hooks so run_bass_kernel_spmd(trace=True) works
    in containers whose antenv lacks it. Harmless if tracing is never used."""
    try:
        import antenv.axon_hooks  # noqa: F401
        return
    except ImportError:
        pass
    try:
        from trn_agent_boot.trn_boot import _ntff_profile_via_ctypes
        hook = _ntff_profile_via_ctypes("/opt/axon/libaxon_pjrt.so")
    except Exception:
        hook = None
    mod = types.ModuleType("antenv.axon_hooks")
    mod.get_axon_ntff_profile_hook = lambda: hook
    mod.set_axon_ntff_profile_hook = lambda h: None
    sys.modules["antenv.axon_hooks"] = mod


# ---------------- device graph ----------------

def build_graph(T, NFILL):
    """T = tiles (units) per weight slot; U = NSLOT*T decode units per core.
    NTC = (U + NFILL)*128 owned-token capacity per core."""
    import concourse.bass as bass
    import concourse.mybir as mybir
    from contextlib import ExitStack

    U = NSLOT * T
    NTC = (U + NFILL) * 128
    NCHUNK = NTC // 128
    RR = U * 128            # routed rows (slab region of buf)
    f32 = mybir.dt.float32
    i32 = mybir.dt.int32
    u32 = mybir.dt.uint32
    AF = mybir.ActivationFunctionType
    OP = mybir.AluOpType
    nc = bass.Bass()

    dp = nc.declare_dram_parameter
    rctxT_in = dp("rctxT", [128, KC * RR], f32, isOutput=False)
    noise_in = dp("noiseb", [RR, M], f32, isOutput=False)
    uwt_in = [dp(f"uwt{s}", [128, KC * M], f32, isOutput=False) for s in range(NSLOT)]
    udecb_in = dp("udecb", [1, NSLOT * M], f32, isOutput=False)
    valid_in = dp("valid", [128, U], f32, isOutput=False)
    onesr_in = dp("onesr", [1, 128], f32, isOutput=False)
    upacked_in = [dp(f"upacked{s}", [M, SLAB], f32, isOutput=False)
                  for s in range(NSLOT)]
    orib_in = dp("oribuf", [NTC, SLAB], f32, isOutput=False)
    c0_in = dp("c0col", [NTC], f32, isOutput=False)
    c1_in = dp("c1col", [NTC], f32, isOutput=False)

    psr_out = dp("psr_o", [NTC, D], f32, isOutput=True)
    atk_out = dp("atk_o", [NTC, D], f32, isOutput=True)
    obfw_out = dp("obfw_o", [128, NTC // 128], f32, isOutput=True)
    ent_out = dp("ent_o", [U, U], f32, isOutput=True)

    col1 = lambda ap: ap.rearrange("(a b) -> a b", b=1)

    es = ExitStack()
    sb = lambda nm, shape, dt=f32: es.enter_context(nc.sbuf_tensor(nm, shape, dt))
    ps = lambda nm, shape: es.enter_context(nc.psum_tensor(nm, shape, f32))
    sem = lambda name: es.enter_context(nc.semaphore(name))

    with es:
        rctxT = sb("rctxT_t", [128, KC * RR])
        g_all = sb("gall_t", [128, U * M])
        udecb = sb("udecb_t", [1, NSLOT * M])
        validt = sb("valid_t", [128, U])
        onesr = sb("onesr_t", [1, 128])
        wt = [sb(f"wt{s}", [128, KC * M]) for s in range(NSLOT)]
        e_sb = [sb(f"e{i}", [128, M]) for i in range(2)]
        Z = [sb(f"Z{i}", [128, 1]) for i in range(2)]
        lnz = [sb(f"lnz{i}", [128, 1]) for i in range(2)]
        scratch = sb("scratch_t", [128, M])
        yt = sb("yt_t", [128, M])
        sxe = sb("sxe_t", [128, 1])
        rz = sb("rz_t", [128, 1])
        mx8 = sb("mx8_t", [128, 8])
        idx8 = [sb(f"idx8_{i}", [128, 8], u32) for i in range(2)]
        entcols = sb("entcols_t", [128, U])
        ent_sb = sb("entsb_t", [U, U])
        slab = [sb(f"slab{i}", [128, SLAB]) for i in range(U)]
        ori = [sb(f"ori{i}", [128, SLAB]) for i in range(NCHUNK)]
        c0t = [sb(f"c0t{i}", [128, 1]) for i in range(NCHUNK)]
        c1t = [sb(f"c1t{i}", [128, 1]) for i in range(NCHUNK)]
        t2 = [sb(f"t2_{i}", [128, 2 * D]) for i in range(NCHUNK)]
        wcols = sb("wcols_t", [128, NCHUNK])
        wrow = sb("wrow_t", [1, 512])
        blendt = [sb(f"blend{i}", [128, 2 * D]) for i in range(NCHUNK)]

        NPSUM = 3
        psum = [ps(f"psum{i}", [128, M]) for i in range(NPSUM)]
        ent_ps = ps("entps_t", [U, U])

        s_const = sem("s_const")
        s_gn = [sem(f"s_gn{i}") for i in range(U)]
        s_wt = [sem(f"s_wt{s}") for s in range(NSLOT)]
        s_pe = sem("s_pe")
        s_act = sem("s_act")
        s_actz = sem("s_actz")
        s_dve = sem("s_dve")
        s_psumfree = sem("s_psumfree")
        s_gat = [sem(f"s_gat{i}") for i in range(U)]
        s_bl = [sem(f"s_bl{i}") for i in range(NCHUNK)]
        s_dveb = sem("s_dveb")
        s_out = sem("s_out")
        s_warm = sem("s_warm")

        N_CONST = 5 * 16  # rctxT x2, udecb, valid, onesr
        OUT_TOTAL = 16 * (2 + 2 * NCHUNK)

        with nc.Block() as block:

            @block.sync
            def _(sync):
                # priority set: weight slot 0, PE consts, unit-0 noise
                H = M // 2
                for kc2 in range(2 * KC):
                    sync.dma_start(wt[0][:, kc2 * H:(kc2 + 1) * H],
                                   uwt_in[0][:, kc2 * H:(kc2 + 1) * H]
                                   ).then_inc(s_wt[0], 16)
                half = (KC * RR) // 2
                sync.dma_start(rctxT[:, :half],
                               rctxT_in[:, :half]).then_inc(s_const, 16)
                sync.dma_start(rctxT[:, half:],
                               rctxT_in[:, half:]).then_inc(s_const, 16)
                sync.dma_start(udecb[:], udecb_in[:]).then_inc(s_const, 16)
                sync.dma_start(validt[:], valid_in[:]).then_inc(s_const, 16)
                sync.dma_start(onesr[:], onesr_in[:]).then_inc(s_const, 16)
                sync.dma_start(g_all[:, 0:M],
                               noise_in[0:128, :]).then_inc(s_gn[0], 16)
                # defer the rest until the priority set owns the wires
                sync.wait_ge(s_wt[0], 32 * KC)
                sync.wait_ge(s_const, N_CONST)
                for u in range(1, T):
                    sync.dma_start(g_all[:, u * M:(u + 1) * M],
                                   noise_in[u * 128:(u + 1) * 128, :]
                                   ).then_inc(s_gn[u], 16)
                for s in range(1, NSLOT):
                    for kc2 in range(2 * KC):
                        sync.dma_start(wt[s][:, kc2 * H:(kc2 + 1) * H],
                                       uwt_in[s][:, kc2 * H:(kc2 + 1) * H]
                                       ).then_inc(s_wt[s], 16)
                    for u in range(s * T, (s + 1) * T):
                        sync.dma_start(g_all[:, u * M:(u + 1) * M],
                                       noise_in[u * 128:(u + 1) * 128, :]
                                       ).then_inc(s_gn[u], 16)
                for c in range(NCHUNK):
                    sl = slice(c * 128, (c + 1) * 128)
                    sync.dma_start(ori[c][:], orib_in[sl, :]).then_inc(s_bl[c], 16)
                    sync.dma_start(c0t[c][:], col1(c0_in[sl])).then_inc(s_bl[c], 16)
                    sync.dma_start(c1t[c][:], col1(c1_in[sl])).then_inc(s_bl[c], 16)
                # outputs (in blend completion order)
                blend_order = list(range(U, NCHUNK)) + list(range(U))
                for bi, c in enumerate(blend_order):
                    sl = slice(c * 128, (c + 1) * 128)
                    sync.wait_ge(s_dveb, bi + 1)
                    sync.dma_start(psr_out[sl, :], blendt[c][:, 0:D]).then_inc(s_out, 16)
                    sync.dma_start(atk_out[sl, :], blendt[c][:, D:2 * D]).then_inc(s_out, 16)
                sync.wait_ge(s_dveb, NCHUNK)
                sync.dma_start(obfw_out[:], wcols[:]).then_inc(s_out, 16)
                sync.wait_ge(s_dve, U + 1)
                sync.dma_start(ent_out[:], ent_sb[:]).then_inc(s_out, 16)
                sync.wait_ge(s_out, OUT_TOTAL)

            @block.tensor
            def _(pe):
                pe.wait_ge(s_const, N_CONST)
                for u in range(U):
                    s = u // T
                    if u % T == 0:
                        pe.wait_ge(s_wt[s], 32 * KC)
                    if u >= NPSUM:
                        pe.wait_ge(s_act, u - NPSUM + 1)
                        pe.wait_ge(s_psumfree, u - NPSUM + 1)
                    pb = psum[u % NPSUM]
                    for nh in range(2):
                        out_ap = pb[:, nh * 512:(nh + 1) * 512]
                        for kc in range(KC):
                            pe.matmul(
                                out=out_ap,
                                lhsT=rctxT[:, kc * RR + u * 128: kc * RR + (u + 1) * 128],
                                rhs=wt[s][:, kc * M + nh * 512: kc * M + (nh + 1) * 512],
                                start=(kc == 0), stop=False)
                        mm = pe.matmul(
                            out=out_ap,
                            lhsT=onesr[:],
                            rhs=udecb[0:1, s * M + nh * 512: s * M + (nh + 1) * 512],
                            start=False, stop=True)
                        if nh == 1:
                            mm.then_inc(s_pe, 1)
                # entropy partials: diag of entcols^T @ valid
                pe.wait_ge(s_dve, U)
                pe.matmul(out=ent_ps[:], lhsT=entcols[:], rhs=validt[:],
                          start=True, stop=True).then_inc(s_pe, 1)

            @block.scalar
            def _(act):
                for u in range(U):
                    act.wait_ge(s_pe, u + 1)
                    if u >= 2:
                        act.wait_ge(s_dve, u - 1)
                    act.activation(e_sb[u % 2][:], psum[u % NPSUM][:], AF.Exp,
                                   accum_out=Z[u % 2][:]).then_inc(s_actz, 1)
                    act.wait_ge(s_actz, u + 1)
                    act.activation(lnz[u % 2][:], Z[u % 2][:], AF.Ln).then_inc(s_act, 1)

            @block.vector
            def _(dve):
                # filler-chunk blends depend only on their loads
                for c in range(U, NCHUNK):
                    dve.wait_ge(s_bl[c], 48)
                    dve.tensor_scalar_mul(t2[c][:], ori[c][:, 0:2 * D], c1t[c][:])
                    dve.tensor_copy(wcols[:, c:c + 1], ori[c][:, 2 * D:2 * D + 1])
                    dve.drain()
                    dve.scalar_tensor_tensor(
                        out=blendt[c][:], in0=ori[c][:, 0:2 * D],
                        scalar=c0t[c][:], in1=t2[c][:],
                        op0=OP.mult, op1=OP.add).then_inc(s_dveb, 1)
                dve.wait_ge(s_const, N_CONST)
                for u in range(U):
                    dve.wait_ge(s_act, u + 1)
                    dve.wait_ge(s_gn[u], 16)
                    if u >= 2:
                        # idx8 slot reuse: gather of u-2 has consumed it
                        dve.wait_ge(s_gat[u - 2], 16)
                    pb = psum[u % NPSUM]
                    dve.tensor_tensor(out=yt[:], in0=pb[:],
                                      in1=g_all[:, u * M:(u + 1) * M], op=OP.add)
                    dve.drain()
                    dve.max(mx8[:], yt[:])
                    # e*x reads psum; independent of max8 -> no drain between
                    dve.tensor_tensor(out=scratch[:], in0=e_sb[u % 2][:],
                                      in1=pb[:], op=OP.mult).then_inc(s_psumfree, 1)
                    dve.drain()
                    dve.max_index(idx8[u % 2][:], mx8[:], yt[:])
                    dve.tensor_reduce(out=sxe[:], in_=scratch[:],
                                      axis=mybir.AxisListType.X, op=OP.add)
                    dve.reciprocal(rz[:], Z[u % 2][:])
                    dve.drain()
                    # entcols[:, u] = sxe/Z - lnZ = -(ent_tok); host negates
                    dve.scalar_tensor_tensor(
                        out=entcols[:, u:u + 1], in0=sxe[:], scalar=rz[:],
                        in1=lnz[u % 2][:], op0=OP.mult,
                        op1=OP.subtract).then_inc(s_dve, 1)
                    if u >= 2:
                        c = u - 2
                        dve.wait_ge(s_bl[c], 48)
                        dve.wait_ge(s_gat[c], 16)
                        dve.tensor_scalar_mul(t2[c][:], slab[c][:, 0:2 * D],
                                              c1t[c][:])
                        dve.tensor_copy(wcols[:, c:c + 1],
                                        slab[c][:, 2 * D:2 * D + 1])
                        dve.drain()
                        dve.scalar_tensor_tensor(
                            out=blendt[c][:], in0=ori[c][:, 0:2 * D],
                            scalar=c0t[c][:], in1=t2[c][:],
                            op0=OP.mult, op1=OP.add).then_inc(s_dveb, 1)
                # remaining routed blends
                for c in range(max(U - 2, 0), U):
                    dve.wait_ge(s_bl[c], 48)
                    dve.wait_ge(s_gat[c], 16)
                    dve.tensor_scalar_mul(t2[c][:], slab[c][:, 0:2 * D], c1t[c][:])
                    dve.tensor_copy(wcols[:, c:c + 1], slab[c][:, 2 * D:2 * D + 1])
                    dve.drain()
                    dve.scalar_tensor_tensor(
                        out=blendt[c][:], in0=ori[c][:, 0:2 * D],
                        scalar=c0t[c][:], in1=t2[c][:],
                        op0=OP.mult, op1=OP.add).then_inc(s_dveb, 1)
                # ent psum -> sbuf
                dve.wait_ge(s_pe, U + 1)
                dve.tensor_copy(ent_sb[:], ent_ps[:]).then_inc(s_dve, 1)

            @block.gpsimd
            def _(gp):
                for u in range(U):
                    s = u // T
                    gp.wait_ge(s_dve, u + 1)
                    gp.indirect_dma_start(
                        out=slab[u][:], out_offset=None, in_=upacked_in[s][:],
                        in_offset=bass.IndirectOffsetOnAxis(
                            ap=idx8[u % 2][:, 0:1], axis=0),
                    ).then_inc(s_gat[u], 16)

    return nc


# ---------------- host side ----------------

def _jax_cpu():
    import jax
    return jax, jax.local_devices(backend="cpu")[0]


def _host_random_and_cpy(ctx_f, W1, b1, W2, b2, pri):
    """Reproduce (bit-exact, jax CPU eager) the reference's random streams and
    the copy-gate head. Returns gumbel noise per POS, cpy columns, logp2[:,1]."""
    jax, cpu = _jax_cpu()
    import jax.numpy as jnp
    with jax.default_device(cpu):
        gkey = jax.random.key(42)
        gs = []
        for p in range(P):
            kp = jax.random.fold_in(gkey, p)
            u = jax.random.uniform(kp, (N, M), jnp.float32, 1e-6, 1.0 - 1e-6)
            gs.append(np.asarray(-jnp.log(-jnp.log(u))))
        h = jax.nn.relu(jnp.asarray(ctx_f) @ jnp.asarray(W1) + jnp.asarray(b1))
        pcpy = jax.nn.log_softmax(h @ jnp.asarray(W2) + jnp.asarray(b2), axis=-1)
        pcpy = jnp.where(jnp.asarray(pri)[:, None],
                         jnp.array([0.0, 1.0], pcpy.dtype), pcpy)
        k2 = jax.random.fold_in(gkey, 999)
        u2 = jax.random.uniform(k2, pcpy.shape, pcpy.dtype, 1e-6, 1.0 - 1e-6)
        g2 = -jnp.log(-jnp.log(u2))
        y = jax.nn.softmax((pcpy + g2) / 1.0, axis=-1)
        idx2 = jnp.argmax(y, axis=-1)
        y_hard = jax.nn.one_hot(idx2, 2, dtype=y.dtype)
        cpy = y_hard + y - jax.lax.stop_gradient(y)
        logp2 = jax.nn.log_softmax(pcpy, axis=-1)
        return gs, np.asarray(cpy), np.asarray(logp2[:, 1])


def host_prep(inputs):
    ctx = np.asarray(inputs["ctx"], np.float32).reshape(N, HS)
    dec_W = np.asarray(inputs["dec_W"], np.float32)
    dec_Wt = np.ascontiguousarray(
        dec_W.reshape(P, KC, 128, M).transpose(0, 2, 1, 3).reshape(P, 128, KC * M))
    dec_b = np.ascontiguousarray(np.asarray(inputs["dec_b"], np.float32))
    psr_w = np.asarray(inputs["psr_weight"], np.float32)
    atk_w = np.asarray(inputs["atk_weight"], np.float32)
    W1 = np.asarray(inputs["cpy_W1"], np.float32)
    b1 = np.asarray(inputs["cpy_b1"], np.float32)
    W2 = np.asarray(inputs["cpy_W2"], np.float32)
    b2 = np.asarray(inputs["cpy_b2"], np.float32)
    word_f = np.asarray(inputs["inp_word"]).reshape(N).astype(np.int32)
    pos_f = np.asarray(inputs["inp_pos"]).reshape(N).astype(np.int32)
    mask_f = np.asarray(inputs["inp_mask"]).reshape(N).astype(bool)
    words = np.asarray(inputs["words"]).astype(np.int32)
    lut = np.asarray(inputs["lut"]).astype(np.int32)

    pri = pos_f < 4
    gs, cpy, logp2_1 = _host_random_and_cpy(ctx, W1, b1, W2, b2, pri)
    c0_all = cpy[:, 0].astype(np.float32)
    c1_all = cpy[:, 1].astype(np.float32)

    packed = np.zeros((P, M, SLAB), np.float32)
    for p in range(P):
        packed[p, :, 0:D] = psr_w[words[p]]
        packed[p, :, D:2 * D] = atk_w[words[p]]
        packed[p, :, 2 * D] = words[p].astype(np.float32)

    # ---- expert dispatch: slots of <= T*128 tokens per POS ----
    tok_by_p = [np.where(pos_f == p)[0].astype(np.int32) for p in range(P)]
    cnt_p = np.array([len(t) for t in tok_by_p])
    nonrouted = np.where(pos_f >= P)[0].astype(np.int32)

    T = 1
    while True:
        SLOT_TOK = T * 128
        slots = []  # (p, token_ids)
        for p in range(P):
            t = tok_by_p[p]
            for i in range(0, max(len(t), 1), SLOT_TOK):
                slots.append((p, t[i:i + SLOT_TOK]))
        if len(slots) <= NCORES * NSLOT:
            break
        T += 1
    while len(slots) < NCORES * NSLOT:
        slots.append((0, np.zeros((0,), np.int32)))
    # deal the biggest slots across cores snake-wise for balance
    order = np.argsort([-len(s[1]) for s in slots])
    core_slots = [[] for _ in range(NCORES)]
    for i, si in enumerate(order):
        k = i % NCORES if (i // NCORES) % 2 == 0 else NCORES - 1 - (i % NCORES)
        core_slots[k].append(slots[si])
    U = NSLOT * T

    total_fill = len(nonrouted)
    NFILL = (total_fill + 128 * NCORES - 1) // (128 * NCORES)
    NTC = (NSLOT * T + NFILL) * 128

    in_maps = []
    perm = np.full((NCORES, NTC), -1, np.int64)  # slot -> global token id
    unit_p = np.full((NCORES, U), -1, np.int64)  # (core, unit) -> p
    fill_ptr = 0
    for k in range(NCORES):
        cs = core_slots[k]
        rctxT = np.zeros((KC, 128, U * 128), np.float32)
        noiseb = np.zeros((U * 128, M), np.float32)
        validf = np.zeros(U * 128, np.float32)   # flat (unit-major)
        uwt = np.zeros((NSLOT, 128, KC * M), np.float32)
        udecb = np.zeros((NSLOT, M), np.float32)
        upk = np.zeros((NSLOT, M, SLAB), np.float32)
        for s in range(NSLOT):
            if s < len(cs):
                p, toks = cs[s]
                uwt[s] = dec_Wt[p]
                udecb[s] = dec_b[p]
                upk[s] = packed[p]
                nn = len(toks)
                if nn:
                    base = s * T * 128
                    rctxT[:, :, base:base + nn] = ctx[toks].T.reshape(KC, 128, nn)
                    noiseb[base:base + nn] = gs[p][toks]
                    validf[base:base + nn] = 1.0
                for t_i in range(T):
                    unit_p[k, s * T + t_i] = p
        # valid as [128, U]: row i, col u = validf[u*128 + i]
        valid = validf.reshape(U, 128).T.copy()
        RRr = U * 128
        # blend-row-aligned token placement: row i (< RRr) = routed slot i
        row_tok = np.full(NTC, -1, np.int64)
        pos_r = 0
        for s in range(NSLOT):
            if s < len(cs):
                p, toks = cs[s]
                base = s * T * 128
                row_tok[base:base + len(toks)] = toks
        room = NTC - RRr
        takef = min(room, total_fill - fill_ptr)
        fills = nonrouted[fill_ptr:fill_ptr + takef]
        fill_ptr += takef
        row_tok[RRr:RRr + len(fills)] = fills
        perm[k] = row_tok
        mrow = row_tok >= 0
        ridx = row_tok[mrow]

        orib = np.zeros((NTC, SLAB), np.float32)
        orib[mrow, 0:D] = psr_w[word_f[ridx]]
        orib[mrow, D:2 * D] = atk_w[word_f[ridx]]
        orib[mrow, 2 * D] = word_f[ridx].astype(np.float32)
        c0c = np.ones(NTC, np.float32)
        c1c = np.zeros(NTC, np.float32)
        c0c[mrow] = c0_all[ridx]
        c1c[mrow] = c1_all[ridx]

        im = {
            "rctxT": np.ascontiguousarray(
                rctxT.transpose(1, 0, 2)).reshape(128, KC * U * 128),
            "noiseb": noiseb,
            "udecb": udecb.reshape(1, NSLOT * M),
            "valid": valid, "oribuf": orib,
            "c0col": c0c, "c1col": c1c,
            "onesr": np.ones((1, 128), np.float32),
        }
        for s in range(NSLOT):
            im[f"uwt{s}"] = uwt[s]
            im[f"upacked{s}"] = upk[s]
        in_maps.append(im)

    assert fill_ptr == total_fill, "non-routed tokens did not all fit"
    state = dict(word_f=word_f, pos_f=pos_f, mask_f=mask_f, lut=lut,
                 c0_all=c0_all, c1_all=c1_all, logp2_1=logp2_1, pri=pri,
                 perm=perm, unit_p=unit_p, cnt_p=cnt_p)
    return in_maps, (T, NFILL), state


def epilogue(results, state):
    word_f = state["word_f"]; pos_f = state["pos_f"]; mask_f = state["mask_f"]
    lut = state["lut"]; c0_all = state["c0_all"]; c1_all = state["c1_all"]
    logp2_1 = state["logp2_1"]; pri = state["pri"]
    perm = state["perm"]; unit_p = state["unit_p"]; cnt_p = state["cnt_p"]

    psr_out = np.zeros((N, D), np.float32)
    atk_out = np.zeros((N, D), np.float32)
    obfw = np.zeros(N, np.float32)
    ent_by_p = np.zeros(P, np.float64)
    for k in range(NCORES):
        ids = perm[k]
        m = ids >= 0
        psr_out[ids[m]] = results[k]["psr_o"][m]
        atk_out[ids[m]] = results[k]["atk_o"][m]
        obfw[ids[m]] = results[k]["obfw_o"].T.reshape(-1)[m]
        diag = -np.diagonal(results[k]["ent_o"])
        for u_i in range(len(diag)):
            p = unit_p[k, u_i]
            if p >= 0:
                ent_by_p[p] += float(diag[u_i])

    wf = word_f.astype(np.float32)
    out_word = (wf * c0_all + obfw * c1_all).astype(np.int32)
    obf_char = lut[out_word]

    cpy_mask = (c0_all == np.float32(1.0)) & mask_f
    obf_base = pos_f < P
    obf_mask = obf_base & ~cpy_mask
    pri_mask = pri.copy()

    entropy = np.float32(0.0)
    for p in range(P):
        if cnt_p[p] > 0:
            ent_p = np.float32(ent_by_p[p] / float(cnt_p[p] * M))
            entropy = np.float32(entropy + ent_p)
    ent_loss = np.float32(-entropy)

    noroot = mask_f.reshape(B, L).copy()
    noroot[:, 0] = False
    noroot = noroot.reshape(N)
    nr_cnt = np.float32(max(int(noroot.sum()), 1))
    cpy_loss = np.float32(-(logp2_1[noroot].astype(np.float64).sum()) / nr_cnt)

    return (out_word.reshape(B, L),
            obf_char.reshape(B, L, CHAR_LEN),
            psr_out.reshape(B, L, D),
            atk_out.reshape(B, L, D),
            obf_mask.reshape(B, L),
            cpy_mask.reshape(B, L),
            pri_mask.reshape(B, L),
            cpy_loss, ent_loss)


def kernel(**inputs):
    _install_axon_profile_hook()
    from concourse.bass_utils import run_bass_kernel_spmd

    in_maps, (T, NFILL), state = host_prep(inputs)
    nc = build_graph(T, NFILL)
    trace = bool(os.environ.get("KERNEL_TRACE"))
    run_kwargs = {}
    if trace:
        import concourse.bass_utils as _bu
        _bu.upload_artifacts = lambda d: d  # zero-egress container
        run_kwargs = {"trace": True,
                      "tmpdir": os.environ.get("KERNEL_TRACE_DIR") or None}
    res = run_bass_kernel_spmd(nc, in_maps, core_ids=list(range(NCORES)),
                               **run_kwargs)
    global LAST_EXEC_NS
    LAST_EXEC_NS = res.exec_time_ns
    return epilogue(res.results, state)


# revision 40
# speedup vs baseline: 1.2536x; 1.2536x over previous
"""Trainium2 Bass kernel for nn_AlltagCopyCtxGenerator (moe_routing).

Expert-parallel sharding with host-side token shuffle (no collectives):
  - Tokens are assigned to cores BY POS GROUP (MoE expert dispatch done on the
    host): each core owns the tokens of the <=2 POS decoders it hosts, plus a
    share of the non-decoded (pos>=12) tokens. Each core therefore reads only
    its own ~2 decoder weight matrices (4MB instead of 24MB).
  - Host: bit-exact jax-CPU reproduction of the gumbel noise streams and the
    tiny copy-gate head (whose `==1.0` / int-cast semantics depend on exact
    float bits), routing tables, packed per-POS gather tables
    [psr_row | atk_row | word], weight/ctx layout transforms.
  - Device (per core, SPMD, raw bass): per decode unit (128 routed tokens),
    f32 matmul against the unit's dec_W (+rank-1 bias), fused exp/accum for
    entropy, gumbel argmax via max/max_index, indirect-DMA gather of packed
    embedding rows, then the copy-gate blend psr/atk = ori*c0 + obf*c1.
  - Host epilogue: inverse shuffle, luck-exact out_word/obf_char/masks/losses.
"""
import os
import sys
import types
import numpy as np

# ---------------- problem constants (hardcoded per spec) ----------------
B, L, HS = 32, 128, 512
P, M = 12, 1024
V, D = 50000, 128
CHAR_LEN = 16
N = B * L
NCORES = 8
SLAB = 260  # psr[128] | atk[128] | word | pad
KC = HS // 128  # contraction chunks
NSLOT = 4       # weight slots per core

LAST_EXEC_NS = None  # filled by kernel() when KERNEL_TRACE is set


def _install_axon_profile_hook():
    """Provide antenv.axon_hooks so run_bass_kernel_spmd(trace=True) works
    in containers whose antenv lacks it. Harmless if tracing is never used."""
    try:
        import antenv.axon_hooks  # noqa: F401
        return
    except ImportError:
        pass
    try:
        from trn_agent_boot.trn_boot import _ntff_profile_via_ctypes
        hook = _ntff_profile_via_ctypes("/opt/axon/libaxon_pjrt.so")
    except Exception:
        hook = None
    mod = types.ModuleType("antenv.axon_hooks")
    mod.get_axon_ntff_profile_hook = lambda: hook
    mod.set_axon_ntff_profile_hook = lambda h: None
    sys.modules["antenv.axon_hooks"] = mod


# ---------------- device graph ----------------

def build_graph(T, NFILL):
    """T = tiles (units) per weight slot; U = NSLOT*T decode units per core.
    NTC = (U + NFILL)*128 owned-token capacity per core."""
    import concourse.bass as bass
    import concourse.mybir as mybir
    from contextlib import ExitStack

    U = NSLOT * T
    NTC = (U + NFILL) * 128
    NCHUNK = NTC // 128
    RR = U * 128            # routed rows (slab region of buf)
    f32 = mybir.dt.float32
    i32 = mybir.dt.int32
    u32 = mybir.dt.uint32
    AF = mybir.ActivationFunctionType
    OP = mybir.AluOpType
    nc = bass.Bass()

    dp = nc.declare_dram_parameter
    rctxT_in = dp("rctxT", [128, KC * RR], f32, isOutput=False)
    noise_in = dp("noiseb", [RR, M], f32, isOutput=False)
    uwt_in = [dp(f"uwt{s}", [128, KC * M], f32, isOutput=False) for s in range(NSLOT)]
    udecb_in = dp("udecb", [1, NSLOT * M], f32, isOutput=False)
    valid_in = dp("valid", [128, U], f32, isOutput=False)
    onesr_in = dp("onesr", [1, 128], f32, isOutput=False)
    upacked_in = [dp(f"upacked{s}", [M, SLAB], f32, isOutput=False)
                  for s in range(NSLOT)]
    orib_in = dp("oribuf", [NTC, SLAB], f32, isOutput=False)
    c0_in = dp("c0col", [NTC], f32, isOutput=False)
    c1_in = dp("c1col", [NTC], f32, isOutput=False)

    psr_out = dp("psr_o", [NTC, D], f32, isOutput=True)
    atk_out = dp("atk_o", [NTC, D], f32, isOutput=True)
    obfw_out = dp("obfw_o", [128, NTC // 128], f32, isOutput=True)
    ent_out = dp("ent_o", [U, U], f32, isOutput=True)

    col1 = lambda ap: ap.rearrange("(a b) -> a b", b=1)

    es = ExitStack()
    sb = lambda nm, shape, dt=f32: es.enter_context(nc.sbuf_tensor(nm, shape, dt))
    ps = lambda nm, shape: es.enter_context(nc.psum_tensor(nm, shape, f32))
    sem = lambda name: es.enter_context(nc.semaphore(name))

    with es:
        rctxT = sb("rctxT_t", [128, KC * RR])
        g_all = sb("gall_t", [128, U * M])
        udecb = sb("udecb_t", [1, NSLOT * M])
        validt = sb("valid_t", [128, U])
        onesr = sb("onesr_t", [1, 128])
        wt = [sb(f"wt{s}", [128, KC * M]) for s in range(NSLOT)]
        e_sb = [sb(f"e{i}", [128, M]) for i in range(2)]
        Z = [sb(f"Z{i}", [128, 1]) for i in range(2)]
        lnz = [sb(f"lnz{i}", [128, 1]) for i in range(2)]
        scratch = sb("scratch_t", [128, M])
        yt = sb("yt_t", [128, M])
        sxe = sb("sxe_t", [128, 1])
        rz = sb("rz_t", [128, 1])
        mx8 = sb("mx8_t", [128, 8])
        idx8 = [sb(f"idx8_{i}", [128, 8], u32) for i in range(2)]
        entcols = sb("entcols_t", [128, U])
        ent_sb = sb("entsb_t", [U, U])
        slab = [sb(f"slab{i}", [128, SLAB]) for i in range(U)]
        ori = [sb(f"ori{i}", [128, SLAB]) for i in range(NCHUNK)]
        c0t = [sb(f"c0t{i}", [128, 1]) for i in range(NCHUNK)]
        c1t = [sb(f"c1t{i}", [128, 1]) for i in range(NCHUNK)]
        t2 = [sb(f"t2_{i}", [128, 2 * D]) for i in range(NCHUNK)]
        wcols = sb("wcols_t", [128, NCHUNK])
        wrow = sb("wrow_t", [1, 512])
        blendt = [sb(f"blend{i}", [128, 2 * D]) for i in range(NCHUNK)]

        NPSUM = 3
        psum = [ps(f"psum{i}", [128, M]) for i in range(NPSUM)]
        ent_ps = ps("entps_t", [U, U])

        s_const = sem("s_const")
        s_gn = [sem(f"s_gn{i}") for i in range(U)]
        s_wt = [sem(f"s_wt{s}") for s in range(NSLOT)]
        s_pe = sem("s_pe")
        s_act = sem("s_act")
        s_actz = sem("s_actz")
        s_dve = sem("s_dve")
        s_psumfree = sem("s_psumfree")
        s_gat = [sem(f"s_gat{i}") for i in range(U)]
        s_bl = [sem(f"s_bl{i}") for i in range(NCHUNK)]
        s_dveb = sem("s_dveb")
        s_out = sem("s_out")
        s_warm = sem("s_warm")

        N_CONST = 5 * 16  # rctxT x2, udecb, valid, onesr
        OUT_TOTAL = 16 * (2 + 2 * NCHUNK)

        with nc.Block() as block:

            @block.sync
            def _(sync):
                # priority set: weight slot 0, PE consts, unit-0 noise
                H = M // 2
                for kc2 in range(2 * KC):
                    sync.dma_start(wt[0][:, kc2 * H:(kc2 + 1) * H],
                                   uwt_in[0][:, kc2 * H:(kc2 + 1) * H]
                                   ).then_inc(s_wt[0], 16)
                half = (KC * RR) // 2
                sync.dma_start(rctxT[:, :half],
                               rctxT_in[:, :half]).then_inc(s_const, 16)
                sync.dma_start(rctxT[:, half:],
                               rctxT_in[:, half:]).then_inc(s_const, 16)
                sync.dma_start(udecb[:], udecb_in[:]).then_inc(s_const, 16)
                sync.dma_start(validt[:], valid_in[:]).then_inc(s_const, 16)
                sync.dma_start(onesr[:], onesr_in[:]).then_inc(s_const, 16)
                sync.dma_start(g_all[:, 0:M],
                               noise_in[0:128, :]).then_inc(s_gn[0], 16)
                # defer the rest until the priority set owns the wires
                sync.wait_ge(s_wt[0], 32 * KC)
                sync.wait_ge(s_const, N_CONST)
                for u in range(1, T):
                    sync.dma_start(g_all[:, u * M:(u + 1) * M],
                                   noise_in[u * 128:(u + 1) * 128, :]
                                   ).then_inc(s_gn[u], 16)
                # filler-chunk blend inputs next: DVE handles those chunks
                # while the decode pipeline is still filling
                for c in range(U, NCHUNK):
                    sl = slice(c * 128, (c + 1) * 128)
                    sync.dma_start(ori[c][:], orib_in[sl, :]).then_inc(s_bl[c], 16)
                    sync.dma_start(c0t[c][:], col1(c0_in[sl])).then_inc(s_bl[c], 16)
                    sync.dma_start(c1t[c][:], col1(c1_in[sl])).then_inc(s_bl[c], 16)
                for s in range(1, NSLOT):
                    for kc2 in range(2 * KC):
                        sync.dma_start(wt[s][:, kc2 * H:(kc2 + 1) * H],
                                       uwt_in[s][:, kc2 * H:(kc2 + 1) * H]
                                       ).then_inc(s_wt[s], 16)
                    for u in range(s * T, (s + 1) * T):
                        sync.dma_start(g_all[:, u * M:(u + 1) * M],
                                       noise_in[u * 128:(u + 1) * 128, :]
                                       ).then_inc(s_gn[u], 16)
                for c in range(U):
                    sl = slice(c * 128, (c + 1) * 128)
                    sync.dma_start(ori[c][:], orib_in[sl, :]).then_inc(s_bl[c], 16)
                    sync.dma_start(c0t[c][:], col1(c0_in[sl])).then_inc(s_bl[c], 16)
                    sync.dma_start(c1t[c][:], col1(c1_in[sl])).then_inc(s_bl[c], 16)
                # outputs (in blend completion order)
                blend_order = list(range(U, NCHUNK)) + list(range(U))
                for bi, c in enumerate(blend_order):
                    sl = slice(c * 128, (c + 1) * 128)
                    sync.wait_ge(s_dveb, bi + 1)
                    sync.dma_start(psr_out[sl, :], blendt[c][:, 0:D]).then_inc(s_out, 16)
                    sync.dma_start(atk_out[sl, :], blendt[c][:, D:2 * D]).then_inc(s_out, 16)
                sync.wait_ge(s_dveb, NCHUNK)
                sync.dma_start(obfw_out[:], wcols[:]).then_inc(s_out, 16)
                sync.wait_ge(s_dve, U + 1)
                sync.dma_start(ent_out[:], ent_sb[:]).then_inc(s_out, 16)
                sync.wait_ge(s_out, OUT_TOTAL)

            @block.tensor
            def _(pe):
                pe.wait_ge(s_const, N_CONST)
                for u in range(U):
                    s = u // T
                    if u % T == 0:
                        pe.wait_ge(s_wt[s], 32 * KC)
                    if u >= NPSUM:
                        pe.wait_ge(s_act, u - NPSUM + 1)
                        pe.wait_ge(s_psumfree, u - NPSUM + 1)
                    pb = psum[u % NPSUM]
                    for nh in range(2):
                        out_ap = pb[:, nh * 512:(nh + 1) * 512]
                        for kc in range(KC):
                            pe.matmul(
                                out=out_ap,
                                lhsT=rctxT[:, kc * RR + u * 128: kc * RR + (u + 1) * 128],
                                rhs=wt[s][:, kc * M + nh * 512: kc * M + (nh + 1) * 512],
                                start=(kc == 0), stop=False)
                        mm = pe.matmul(
                            out=out_ap,
                            lhsT=onesr[:],
                            rhs=udecb[0:1, s * M + nh * 512: s * M + (nh + 1) * 512],
                            start=False, stop=True)
                        if nh == 1:
                            mm.then_inc(s_pe, 1)
                # entropy partials: diag of entcols^T @ valid
                pe.wait_ge(s_dve, U)
                pe.matmul(out=ent_ps[:], lhsT=entcols[:], rhs=validt[:],
                          start=True, stop=True).then_inc(s_pe, 1)

            @block.scalar
            def _(act):
                for u in range(U):
                    act.wait_ge(s_pe, u + 1)
                    if u >= 2:
                        act.wait_ge(s_dve, u - 1)
                    act.activation(e_sb[u % 2][:], psum[u % NPSUM][:], AF.Exp,
                                   accum_out=Z[u % 2][:]).then_inc(s_actz, 1)
                    act.wait_ge(s_actz, u + 1)
                    act.activation(lnz[u % 2][:], Z[u % 2][:], AF.Ln).then_inc(s_act, 1)

            @block.vector
            def _(dve):
                # filler-chunk blends depend only on their loads
                for c in range(U, NCHUNK):
                    dve.wait_ge(s_bl[c], 48)
                    dve.tensor_scalar_mul(t2[c][:], ori[c][:, 0:2 * D], c1t[c][:])
                    dve.tensor_copy(wcols[:, c:c + 1], ori[c][:, 2 * D:2 * D + 1])
                    dve.drain()
                    dve.scalar_tensor_tensor(
                        out=blendt[c][:], in0=ori[c][:, 0:2 * D],
                        scalar=c0t[c][:], in1=t2[c][:],
                        op0=OP.mult, op1=OP.add).then_inc(s_dveb, 1)
                dve.wait_ge(s_const, N_CONST)
                for u in range(U):
                    dve.wait_ge(s_act, u + 1)
                    dve.wait_ge(s_gn[u], 16)
                    if u >= 2:
                        # idx8 slot reuse: gather of u-2 has consumed it
                        dve.wait_ge(s_gat[u - 2], 16)
                    pb = psum[u % NPSUM]
                    dve.tensor_tensor(out=yt[:], in0=pb[:],
                                      in1=g_all[:, u * M:(u + 1) * M], op=OP.add)
                    dve.drain()
                    dve.max(mx8[:], yt[:])
                    # e*x reads psum; independent of max8 -> no drain between
                    dve.tensor_tensor(out=scratch[:], in0=e_sb[u % 2][:],
                                      in1=pb[:], op=OP.mult).then_inc(s_psumfree, 1)
                    dve.drain()
                    dve.max_index(idx8[u % 2][:], mx8[:], yt[:])
                    dve.tensor_reduce(out=sxe[:], in_=scratch[:],
                                      axis=mybir.AxisListType.X, op=OP.add)
                    dve.reciprocal(rz[:], Z[u % 2][:])
                    dve.drain()
                    # entcols[:, u] = sxe/Z - lnZ = -(ent_tok); host negates
                    dve.scalar_tensor_tensor(
                        out=entcols[:, u:u + 1], in0=sxe[:], scalar=rz[:],
                        in1=lnz[u % 2][:], op0=OP.mult,
                        op1=OP.subtract).then_inc(s_dve, 1)
                    if u >= 2:
                        c = u - 2
                        dve.wait_ge(s_bl[c], 48)
                        dve.wait_ge(s_gat[c], 16)
                        dve.tensor_scalar_mul(t2[c][:], slab[c][:, 0:2 * D],
                                              c1t[c][:])
                        dve.tensor_copy(wcols[:, c:c + 1],
                                        slab[c][:, 2 * D:2 * D + 1])
                        dve.drain()
                        dve.scalar_tensor_tensor(
                            out=blendt[c][:], in0=ori[c][:, 0:2 * D],
                            scalar=c0t[c][:], in1=t2[c][:],
                            op0=OP.mult, op1=OP.add).then_inc(s_dveb, 1)
                # remaining routed blends
                for c in range(max(U - 2, 0), U):
                    dve.wait_ge(s_bl[c], 48)
                    dve.wait_ge(s_gat[c], 16)
                    dve.tensor_scalar_mul(t2[c][:], slab[c][:, 0:2 * D], c1t[c][:])
                    dve.tensor_copy(wcols[:, c:c + 1], slab[c][:, 2 * D:2 * D + 1])
                    dve.drain()
                    dve.scalar_tensor_tensor(
                        out=blendt[c][:], in0=ori[c][:, 0:2 * D],
                        scalar=c0t[c][:], in1=t2[c][:],
                        op0=OP.mult, op1=OP.add).then_inc(s_dveb, 1)
                # ent psum -> sbuf
                dve.wait_ge(s_pe, U + 1)
                dve.tensor_copy(ent_sb[:], ent_ps[:]).then_inc(s_dve, 1)

            @block.gpsimd
            def _(gp):
                for u in range(U):
                    s = u // T
                    gp.wait_ge(s_dve, u + 1)
                    gp.indirect_dma_start(
                        out=slab[u][:], out_offset=None, in_=upacked_in[s][:],
                        in_offset=bass.IndirectOffsetOnAxis(
                            ap=idx8[u % 2][:, 0:1], axis=0),
                    ).then_inc(s_gat[u], 16)

    return nc


# ---------------- host side ----------------

def _jax_cpu():
    import jax
    return jax, jax.local_devices(backend="cpu")[0]


def _host_random_and_cpy(ctx_f, W1, b1, W2, b2, pri):
    """Reproduce (bit-exact, jax CPU eager) the reference's random streams and
    the copy-gate head. Returns gumbel noise per POS, cpy columns, logp2[:,1]."""
    jax, cpu = _jax_cpu()
    import jax.numpy as jnp
    with jax.default_device(cpu):
        gkey = jax.random.key(42)
        gs = []
        for p in range(P):
            kp = jax.random.fold_in(gkey, p)
            u = jax.random.uniform(kp, (N, M), jnp.float32, 1e-6, 1.0 - 1e-6)
            gs.append(np.asarray(-jnp.log(-jnp.log(u))))
        h = jax.nn.relu(jnp.asarray(ctx_f) @ jnp.asarray(W1) + jnp.asarray(b1))
        pcpy = jax.nn.log_softmax(h @ jnp.asarray(W2) + jnp.asarray(b2), axis=-1)
        pcpy = jnp.where(jnp.asarray(pri)[:, None],
                         jnp.array([0.0, 1.0], pcpy.dtype), pcpy)
        k2 = jax.random.fold_in(gkey, 999)
        u2 = jax.random.uniform(k2, pcpy.shape, pcpy.dtype, 1e-6, 1.0 - 1e-6)
        g2 = -jnp.log(-jnp.log(u2))
        y = jax.nn.softmax((pcpy + g2) / 1.0, axis=-1)
        idx2 = jnp.argmax(y, axis=-1)
        y_hard = jax.nn.one_hot(idx2, 2, dtype=y.dtype)
        cpy = y_hard + y - jax.lax.stop_gradient(y)
        logp2 = jax.nn.log_softmax(pcpy, axis=-1)
        return gs, np.asarray(cpy), np.asarray(logp2[:, 1])


def host_prep(inputs):
    ctx = np.asarray(inputs["ctx"], np.float32).reshape(N, HS)
    dec_W = np.asarray(inputs["dec_W"], np.float32)
    dec_Wt = np.ascontiguousarray(
        dec_W.reshape(P, KC, 128, M).transpose(0, 2, 1, 3).reshape(P, 128, KC * M))
    dec_b = np.ascontiguousarray(np.asarray(inputs["dec_b"], np.float32))
    psr_w = np.asarray(inputs["psr_weight"], np.float32)
    atk_w = np.asarray(inputs["atk_weight"], np.float32)
    W1 = np.asarray(inputs["cpy_W1"], np.float32)
    b1 = np.asarray(inputs["cpy_b1"], np.float32)
    W2 = np.asarray(inputs["cpy_W2"], np.float32)
    b2 = np.asarray(inputs["cpy_b2"], np.float32)
    word_f = np.asarray(inputs["inp_word"]).reshape(N).astype(np.int32)
    pos_f = np.asarray(inputs["inp_pos"]).reshape(N).astype(np.int32)
    mask_f = np.asarray(inputs["inp_mask"]).reshape(N).astype(bool)
    words = np.asarray(inputs["words"]).astype(np.int32)
    lut = np.asarray(inputs["lut"]).astype(np.int32)

    pri = pos_f < 4
    gs, cpy, logp2_1 = _host_random_and_cpy(ctx, W1, b1, W2, b2, pri)
    c0_all = cpy[:, 0].astype(np.float32)
    c1_all = cpy[:, 1].astype(np.float32)

    packed = np.zeros((P, M, SLAB), np.float32)
    for p in range(P):
        packed[p, :, 0:D] = psr_w[words[p]]
        packed[p, :, D:2 * D] = atk_w[words[p]]
        packed[p, :, 2 * D] = words[p].astype(np.float32)

    # ---- expert dispatch: slots of <= T*128 tokens per POS ----
    tok_by_p = [np.where(pos_f == p)[0].astype(np.int32) for p in range(P)]
    cnt_p = np.array([len(t) for t in tok_by_p])
    nonrouted = np.where(pos_f >= P)[0].astype(np.int32)

    T = 1
    while True:
        SLOT_TOK = T * 128
        slots = []  # (p, token_ids)
        for p in range(P):
            t = tok_by_p[p]
            for i in range(0, max(len(t), 1), SLOT_TOK):
                slots.append((p, t[i:i + SLOT_TOK]))
        if len(slots) <= NCORES * NSLOT:
            break
        T += 1
    while len(slots) < NCORES * NSLOT:
        slots.append((0, np.zeros((0,), np.int32)))
    # deal the biggest slots across cores snake-wise for balance
    order = np.argsort([-len(s[1]) for s in slots])
    core_slots = [[] for _ in range(NCORES)]
    for i, si in enumerate(order):
        k = i % NCORES if (i // NCORES) % 2 == 0 else NCORES - 1 - (i % NCORES)
        core_slots[k].append(slots[si])
    U = NSLOT * T

    total_fill = len(nonrouted)
    NFILL = (total_fill + 128 * NCORES - 1) // (128 * NCORES)
    NTC = (NSLOT * T + NFILL) * 128

    in_maps = []
    perm = np.full((NCORES, NTC), -1, np.int64)  # slot -> global token id
    unit_p = np.full((NCORES, U), -1, np.int64)  # (core, unit) -> p
    fill_ptr = 0
    for k in range(NCORES):
        cs = core_slots[k]
        rctxT = np.zeros((KC, 128, U * 128), np.float32)
        noiseb = np.zeros((U * 128, M), np.float32)
        validf = np.zeros(U * 128, np.float32)   # flat (unit-major)
        uwt = np.zeros((NSLOT, 128, KC * M), np.float32)
        udecb = np.zeros((NSLOT, M), np.float32)
        upk = np.zeros((NSLOT, M, SLAB), np.float32)
        for s in range(NSLOT):
            if s < len(cs):
                p, toks = cs[s]
                uwt[s] = dec_Wt[p]
                udecb[s] = dec_b[p]
                upk[s] = packed[p]
                nn = len(toks)
                if nn:
                    base = s * T * 128
                    rctxT[:, :, base:base + nn] = ctx[toks].T.reshape(KC, 128, nn)
                    noiseb[base:base + nn] = gs[p][toks]
                    validf[base:base + nn] = 1.0
                for t_i in range(T):
                    unit_p[k, s * T + t_i] = p
        # valid as [128, U]: row i, col u = validf[u*128 + i]
        valid = validf.reshape(U, 128).T.copy()
        RRr = U * 128
        # blend-row-aligned token placement: row i (< RRr) = routed slot i
        row_tok = np.full(NTC, -1, np.int64)
        pos_r = 0
        for s in range(NSLOT):
            if s < len(cs):
                p, toks = cs[s]
                base = s * T * 128
                row_tok[base:base + len(toks)] = toks
        room = NTC - RRr
        takef = min(room, total_fill - fill_ptr)
        fills = nonrouted[fill_ptr:fill_ptr + takef]
        fill_ptr += takef
        row_tok[RRr:RRr + len(fills)] = fills
        perm[k] = row_tok
        mrow = row_tok >= 0
        ridx = row_tok[mrow]

        orib = np.zeros((NTC, SLAB), np.float32)
        orib[mrow, 0:D] = psr_w[word_f[ridx]]
        orib[mrow, D:2 * D] = atk_w[word_f[ridx]]
        orib[mrow, 2 * D] = word_f[ridx].astype(np.float32)
        c0c = np.ones(NTC, np.float32)
        c1c = np.zeros(NTC, np.float32)
        c0c[mrow] = c0_all[ridx]
        c1c[mrow] = c1_all[ridx]

        im = {
            "rctxT": np.ascontiguousarray(
                rctxT.transpose(1, 0, 2)).reshape(128, KC * U * 128),
            "noiseb": noiseb,
            "udecb": udecb.reshape(1, NSLOT * M),
            "valid": valid, "oribuf": orib,
            "c0col": c0c, "c1col": c1c,
            "onesr": np.ones((1, 128), np.float32),
        }
        for s in range(NSLOT):
            im[f"uwt{s}"] = uwt[s]
            im[f"upacked{s}"] = upk[s]
        in_maps.append(im)

    assert fill_ptr == total_fill, "non-routed tokens did not all fit"
    state = dict(word_f=word_f, pos_f=pos_f, mask_f=mask_f, lut=lut,
                 c0_all=c0_all, c1_all=c1_all, logp2_1=logp2_1, pri=pri,
                 perm=perm, unit_p=unit_p, cnt_p=cnt_p)
    return in_maps, (T, NFILL), state


def epilogue(results, state):
    word_f = state["word_f"]; pos_f = state["pos_f"]; mask_f = state["mask_f"]
    lut = state["lut"]; c0_all = state["c0_all"]; c1_all = state["c1_all"]
    logp2_1 = state["logp2_1"]; pri = state["pri"]
    perm = state["perm"]; unit_p = state["unit_p"]; cnt_p = state["cnt_p"]

    psr_out = np.zeros((N, D), np.float32)
    atk_out = np.zeros((N, D), np.float32)
    obfw = np.zeros(N, np.float32)
    ent_by_p = np.zeros(P, np.float64)
    for k in range(NCORES):
        ids = perm[k]
        m = ids >= 0
        psr_out[ids[m]] = results[k]["psr_o"][m]
        atk_out[ids[m]] = results[k]["atk_o"][m]
        obfw[ids[m]] = results[k]["obfw_o"].T.reshape(-1)[m]
        diag = -np.diagonal(results[k]["ent_o"])
        for u_i in range(len(diag)):
            p = unit_p[k, u_i]
            if p >= 0:
                ent_by_p[p] += float(diag[u_i])

    wf = word_f.astype(np.float32)
    out_word = (wf * c0_all + obfw * c1_all).astype(np.int32)
    obf_char = lut[out_word]

    cpy_mask = (c0_all == np.float32(1.0)) & mask_f
    obf_base = pos_f < P
    obf_mask = obf_base & ~cpy_mask
    pri_mask = pri.copy()

    entropy = np.float32(0.0)
    for p in range(P):
        if cnt_p[p] > 0:
            ent_p = np.float32(ent_by_p[p] / float(cnt_p[p] * M))
            entropy = np.float32(entropy + ent_p)
    ent_loss = np.float32(-entropy)

    noroot = mask_f.reshape(B, L).copy()
    noroot[:, 0] = False
    noroot = noroot.reshape(N)
    nr_cnt = np.float32(max(int(noroot.sum()), 1))
    cpy_loss = np.float32(-(logp2_1[noroot].astype(np.float64).sum()) / nr_cnt)

    return (out_word.reshape(B, L),
            obf_char.reshape(B, L, CHAR_LEN),
            psr_out.reshape(B, L, D),
            atk_out.reshape(B, L, D),
            obf_mask.reshape(B, L),
            cpy_mask.reshape(B, L),
            pri_mask.reshape(B, L),
            cpy_loss, ent_loss)


def kernel(**inputs):
    _install_axon_profile_hook()
    from concourse.bass_utils import run_bass_kernel_spmd

    in_maps, (T, NFILL), state = host_prep(inputs)
    nc = build_graph(T, NFILL)
    trace = bool(os.environ.get("KERNEL_TRACE"))
    run_kwargs = {}
    if trace:
        import concourse.bass_utils as _bu
        _bu.upload_artifacts = lambda d: d  # zero-egress container
        run_kwargs = {"trace": True,
                      "tmpdir": os.environ.get("KERNEL_TRACE_DIR") or None}
    res = run_bass_kernel_spmd(nc, in_maps, core_ids=list(range(NCORES)),
                               **run_kwargs)
    global LAST_EXEC_NS
    LAST_EXEC_NS = res.exec_time_ns
    return epilogue(res.results, state)
